# revision 1
# baseline (speedup 1.0000x reference)
"""BERT interaction head on 8 trn2 NeuronCores.

Strategy (data-parallel, CLS-row folding):
  - Batch 16 is sharded 2 sequences per core; each core runs the full head
    for its 2 sequences; host concatenates the 16 scalars.
  - The output only depends on attention query row 0 (the CLS token), so the
    full Q/K/V projections are never materialized:
      scores_h = x @ (wk[:, h] @ q0_h) / sqrt(D)     (K never computed)
      ctx      = diag_blocks((probs @ x) @ wv) + bv  (V never computed)
    The bk term is constant per softmax row and cancels exactly.
  - All matmuls run as float32r (single-pass PE) with fp32 PSUM accumulation.
  - Program order is arranged so seq-0's transpose work overlaps the weight
    DMAs and seq-1's feature load overlaps seq-0's attention.
"""

from contextlib import ExitStack

import numpy as np

import concourse.bacc as bacc
import concourse.bass as bass
import concourse.tile as tile
from concourse import mybir
from concourse._compat import with_exitstack
from concourse.bass_utils import run_bass_kernel_spmd
from concourse.masks import make_identity

F32 = mybir.dt.float32
F32R = mybir.dt.float32r

B, S, H, NH, D, FF = 16, 1024, 768, 12, 64, 3072
N_CORES = 8
BL = B // N_CORES  # 2 sequences per core
HC = H // 128      # 6
SC = S // 128      # 8
FFC = FF // 128    # 24
EPS = 1e-12


def _ap(t, offset, dims):
    return bass.AP(tensor=t, offset=offset, ap=dims)


def _apr(t, offset, dims):
    return bass.AP(tensor=t, offset=offset, ap=dims).bitcast(F32R)


@with_exitstack
def bert_tile_kernel(ctx: ExitStack, tc: tile.TileContext, io: dict, repeat: int = 1):
    for _rep in range(repeat):
        _one_pass(tc, io)


def _one_pass(tc: tile.TileContext, io: dict):
    nc = tc.nc
    feat = io["features"]          # [2, 1024, 768]
    amask = io["attention_mask"]   # [2, 1024]
    out = io["out"]                # [2, 1]

    with ExitStack() as ctx:
        # ---------------- pools (SBUF stack order matters) ----------------
        consts = ctx.enter_context(tc.tile_pool(name="consts", bufs=1))
        pwo = ctx.enter_context(tc.tile_pool(name="pwo", bufs=1))
        # FFN weight streams: alive from t=0 so their HWDGE transfers fill
        # every DMA gap during stage 1 (prefetch depth = pool size).
        pw1 = ctx.enter_context(tc.tile_pool(name="pw1", bufs=5))
        pw2 = ctx.enter_context(tc.tile_pool(name="pw2", bufs=6))
        stage1_ctx = ctx.enter_context(ExitStack())
        pwv = stage1_ctx.enter_context(tc.tile_pool(name="pwv", bufs=1))
        px = stage1_ctx.enter_context(tc.tile_pool(name="px", bufs=1))
        pxt0_ctx = stage1_ctx.enter_context(ExitStack())
        pxt = pxt0_ctx.enter_context(tc.tile_pool(name="pxt", bufs=1))

        ppt = ctx.enter_context(tc.tile_pool(name="ppt", bufs=4, space="PSUM"))
        ppm = ctx.enter_context(tc.tile_pool(name="ppm", bufs=2, space="PSUM"))
        pps = ctx.enter_context(tc.tile_pool(name="pps", bufs=2, space="PSUM"))

        # ---------------- identity first (gates all PE transposes) ----------
        ident_f = consts.tile([128, 128], F32)
        make_identity(nc, ident_f)
        ident = consts.tile([128, 128], F32R)
        nc.vector.tensor_copy(out=ident, in_=ident_f)

        ones_f = consts.tile([1, 16], F32)
        nc.vector.memset(ones_f, 1.0)
        ones_row = consts.tile([1, 16], F32R)
        nc.vector.tensor_copy(out=ones_row, in_=ones_f)

        # f0 rows (CLS features) as f32r, plus transposed copy
        f0_2 = consts.tile([BL, H], F32R)
        nc.sync.dma_start(
            out=f0_2, in_=_apr(feat.tensor, 0, [[S * H, BL], [1, H]])
        )
        f0T = consts.tile([128, HC, BL], F32R)
        for c in range(HC):
            pt = ppt.tile([128, BL], F32R, name="pt", tag="pt")
            nc.tensor.transpose(pt[:, :], f0_2[:, c * 128:(c + 1) * 128], ident[0:BL, 0:BL])
            nc.vector.tensor_copy(out=f0T[:, c, :], in_=pt[:, :])

        def load_row_r(name, n):  # [1, n] fp32 dram -> f32r sbuf row
            t = consts.tile([1, n], F32R, name=f"{name}_row")
            nc.sync.dma_start(out=t, in_=_apr(io[name].tensor, 0, [[0, 1], [1, n]]))
            return t

        bq_row = load_row_r("bq", H)

        # bv and wm as column stacks via PE transpose (2-wide: fp32r matmul
        # requires even innermost dims, so transpose duplicated 2-row inputs)
        bv_2 = consts.tile([BL, H], F32R)
        nc.sync.dma_start(out=bv_2, in_=_apr(io["bv"].tensor, 0, [[0, BL], [1, H]]))

        # feature load for seq 0 (HWDGE with f32r bitcast — a bit copy)
        x0 = px.tile([128, SC, H], F32R, name="x0")
        for sc in range(SC):
            nc.sync.dma_start(
                out=x0[:, sc, :],
                in_=_apr(feat.tensor, sc * 128 * H, [[H, 128], [1, H]]),
            )
        bvT = consts.tile([128, HC, BL], F32R)
        for c in range(HC):
            pt = ppt.tile([128, BL], F32R, name="pt", tag="pt")
            nc.tensor.transpose(pt[:, :], bv_2[:, c * 128:(c + 1) * 128], ident[0:BL, 0:BL])
            nc.vector.tensor_copy(out=bvT[:, c, :], in_=pt[:, :])

        # stage-1 outputs
        ctxT = consts.tile([128, HC, BL], F32R)
        zeros_f = consts.tile([128, NH * BL], F32)
        nc.vector.memset(zeros_f, 0.0)
        q0bd = consts.tile([128, HC, NH * BL], F32R)
        for _c in range(HC):
            nc.vector.tensor_copy(out=q0bd[:, _c, :], in_=zeros_f)
        U_sb = consts.tile([128, HC, NH * BL], F32R)

        wv_sb = pwv.tile([128, HC, H], F32R)
        nc.gpsimd.dma_start(
            out=wv_sb, in_=_ap(io["wv"].tensor, 0, [[H, 128], [128 * H, HC], [1, H]])
        )
        # wo resident early so the row chain can start without waiting.
        # wv/wo ride the gpsimd (SWDGE) path: separate queue from the
        # latency-critical sync loads (x/wq/wk).
        wo_sb = pwo.tile([128, HC, H], F32R)
        nc.gpsimd.dma_start(
            out=wo_sb, in_=_ap(io["wo"].tensor, 0, [[H, 128], [128 * H, HC], [1, H]])
        )

        # ---- xT for seq 0: pure PE/DVE work overlapping the weight DMAs
        def build_xT(x_nat, pool=pxt):
            xT = pool.tile([128, HC, S], F32R, name="xT")
            for hc in range(HC):
                for sc in range(SC):
                    pt = ppt.tile([128, 128], F32R, name="pt", tag="pt")
                    nc.tensor.transpose(
                        pt[:, :], x_nat[:, sc, hc * 128:(hc + 1) * 128], ident[:, :]
                    )
                    dst = xT[:, hc, sc * 128:(sc + 1) * 128]
                    if (hc * SC + sc) % 2 == 0:
                        nc.vector.tensor_copy(out=dst, in_=pt[:, :])
                    else:
                        nc.scalar.activation(
                            out=dst, in_=pt[:, :],
                            func=mybir.ActivationFunctionType.Copy,
                        )
            return xT

        xT0 = build_xT(x0)

        # ---------------- q0 / wkT / U ----------------
        with tc.tile_pool(name="pwk_t", bufs=1) as pwkT:
            wkT_sb = pwkT.tile([128, HC, H], F32R)

            with tc.tile_pool(name="pwk_n", bufs=1) as pwkn:
                wk_nat = pwkn.tile([128, HC, H], F32R)
                for c in range(HC):
                    nc.sync.dma_start(
                        out=wk_nat[:, c, :],
                        in_=_apr(io["wk"].tensor, c * 128 * H, [[H, 128], [1, H]]),
                    )

                with tc.tile_pool(name="pwq", bufs=2) as pwq:
                    ps_q0 = [ppm.tile([BL, 512], F32, name="mm", tag="mm"),
                             ppm.tile([BL, 256], F32, name="mm", tag="mm")]
                    for c in range(HC):
                        wq_c = pwq.tile([128, H], F32R, name="wq_c")
                        nc.sync.dma_start(
                            out=wq_c,
                            in_=_apr(io["wq"].tensor, c * 128 * H, [[H, 128], [1, H]]),
                        )
                        nc.tensor.matmul(ps_q0[0][:, :], f0T[:, c, :], wq_c[:, 0:512],
                                         start=(c == 0), stop=False)
                        nc.tensor.matmul(ps_q0[1][:, :], f0T[:, c, :], wq_c[:, 512:768],
                                         start=(c == 0), stop=False)
                    nc.tensor.matmul(ps_q0[0][:, :], ones_row[0:1, 0:BL], bq_row[0:1, 0:512],
                                     start=False, stop=True)
                    nc.tensor.matmul(ps_q0[1][:, :], ones_row[0:1, 0:BL], bq_row[0:1, 512:768],
                                     start=False, stop=True)
                    q0_sb = consts.tile([BL, H], F32R)
                    nc.vector.tensor_copy(out=q0_sb[:, 0:512], in_=ps_q0[0][:, :])
                    nc.vector.tensor_copy(out=q0_sb[:, 512:768], in_=ps_q0[1][:, :])

                    # q0 block-diagonal, scaled by 1/sqrt(D)
                    # q0bd[p, c, 12*j + head] with head = 2c + p//64
                    for c in range(HC):
                        pt = ppt.tile([128, BL], F32R, name="pt", tag="pt")
                        nc.tensor.transpose(
                            pt[:, :], q0_sb[:, c * 128:(c + 1) * 128],
                            ident[0:BL, 0:BL],
                        )
                        for j in range(BL):
                            nc.vector.tensor_scalar_mul(
                                out=q0bd[0:64, c, NH * j + 2 * c: NH * j + 2 * c + 1],
                                in0=pt[0:64, j:j + 1], scalar1=1.0 / 8.0,
                            )
                            nc.vector.tensor_scalar_mul(
                                out=q0bd[64:128, c, NH * j + 2 * c + 1: NH * j + 2 * c + 2],
                                in0=pt[64:128, j:j + 1], scalar1=1.0 / 8.0,
                            )

                # wkT via PE transposes
                for c in range(HC):      # hh chunk of wk_nat
                    for d in range(HC):  # hd chunk
                        pt = ppt.tile([128, 128], F32R, name="pt", tag="pt")
                        nc.tensor.transpose(
                            pt[:, :], wk_nat[:, c, d * 128:(d + 1) * 128], ident[:, :]
                        )
                        dst = wkT_sb[:, d, c * 128:(c + 1) * 128]
                        if (c * HC + d) % 2 == 0:
                            nc.vector.tensor_copy(out=dst, in_=pt[:, :])
                        else:
                            nc.scalar.activation(
                                out=dst, in_=pt[:, :],
                                func=mybir.ActivationFunctionType.Copy,
                            )

            # U = wk^T-contracted q0bd (both sequences at once)
            for c in range(HC):  # hh chunk (output rows)
                ps_u = ppm.tile([128, NH * BL], F32, name="mm", tag="mm")
                for d in range(HC):  # hd chunk (contraction)
                    nc.tensor.matmul(
                        ps_u[:, :], wkT_sb[:, d, c * 128:(c + 1) * 128], q0bd[:, d, :],
                        start=(d == 0), stop=(d == HC - 1),
                    )
                if c % 2 == 0:
                    nc.vector.tensor_copy(out=U_sb[:, c, :], in_=ps_u[:, :])
                else:
                    nc.scalar.activation(
                        out=U_sb[:, c, :], in_=ps_u[:, :],
                        func=mybir.ActivationFunctionType.Copy,
                    )


        # ---------------- per-sequence attention ----------------
        def scores_softmax(j, xT):
            ps_s = [pps.tile([NH, 512], F32, name="ps_s", tag="ps_s"),
                    pps.tile([NH, 512], F32, name="ps_s", tag="ps_s")]
            for hc in range(HC):
                lhs = U_sb[:, hc, NH * j: NH * (j + 1)]
                nc.tensor.matmul(ps_s[0][:, :], lhs, xT[:, hc, 0:512],
                                 start=(hc == 0), stop=(hc == HC - 1))
                nc.tensor.matmul(ps_s[1][:, :], lhs, xT[:, hc, 512:1024],
                                 start=(hc == 0), stop=(hc == HC - 1))

            mask_bc = consts.tile([NH, S], F32, name="mask_bc", bufs=1)
            nc.sync.dma_start(
                out=mask_bc, in_=_ap(amask.tensor, j * S, [[0, NH], [1, S]])
            )
            scores = consts.tile([NH, S], F32, name="scores", bufs=1)
            nc.vector.tensor_add(out=scores[:, 0:512], in0=ps_s[0][:, :], in1=mask_bc[:, 0:512])
            nc.vector.tensor_add(out=scores[:, 512:1024], in0=ps_s[1][:, :], in1=mask_bc[:, 512:1024])

            negmax = consts.tile([NH, 1], F32, name="negmax", bufs=1)
            nc.vector.reduce_max(out=negmax, in_=scores, axis=mybir.AxisListType.X, negate=True)
            sumexp = consts.tile([NH, 1], F32, name="sumexp", bufs=1)
            probs = consts.tile([NH, S], F32R, name="probs", bufs=1)
            nc.scalar.activation(
                out=probs, in_=scores, func=mybir.ActivationFunctionType.Exp,
                bias=negmax, scale=1.0, accum_out=sumexp,
            )
            rec = consts.tile([NH, 1], F32, name="rec", bufs=1)
            nc.vector.reciprocal(out=rec, in_=sumexp)
            nc.vector.tensor_scalar_mul(out=probs, in0=probs, scalar1=rec)

            probsT = consts.tile([128, SC, NH], F32R, name="probsT", bufs=1)
            for sc in range(SC):
                pt = ppt.tile([128, NH], F32R, name="pt", tag="pt")
                nc.tensor.transpose(
                    pt[:, :], probs[:, sc * 128:(sc + 1) * 128], ident[0:NH, 0:NH]
                )
                if sc % 2 == 0:
                    nc.vector.tensor_copy(out=probsT[:, sc, :], in_=pt[:, :])
                else:
                    nc.scalar.activation(
                        out=probsT[:, sc, :], in_=pt[:, :],
                        func=mybir.ActivationFunctionType.Copy,
                    )
            return probsT

        def yt_zt(j, x_nat, probsT):
            # Y^T [hh, 12] = sum_s x^T probs^T  (lhsT = x blocks)
            YT_sb = consts.tile([128, HC, NH], F32R, name="YT_sb", bufs=1)
            for hc in range(HC):
                ps_y = ppm.tile([128, NH], F32, name="mm", tag="mm")
                for sc in range(SC):
                    nc.tensor.matmul(
                        ps_y[:, :], x_nat[:, sc, hc * 128:(hc + 1) * 128],
                        probsT[:, sc, :], start=(sc == 0), stop=(sc == SC - 1),
                    )
                if hc % 2 == 0:
                    nc.vector.tensor_copy(out=YT_sb[:, hc, :], in_=ps_y[:, :])
                else:
                    nc.scalar.activation(
                        out=YT_sb[:, hc, :], in_=ps_y[:, :],
                        func=mybir.ActivationFunctionType.Copy,
                    )

            # Z^T chunks [hd, 12]; diag-extract + bv -> ctxT[:, :, j]
            for hd in range(HC):
                ps_z = ppm.tile([128, NH], F32, name="mm", tag="mm")
                for hc in range(HC):
                    nc.tensor.matmul(
                        ps_z[:, :], wv_sb[:, hc, hd * 128:(hd + 1) * 128],
                        YT_sb[:, hc, :], start=(hc == 0), stop=(hc == HC - 1),
                    )
                nc.vector.tensor_add(
                    out=ctxT[0:64, hd, j:j + 1],
                    in0=ps_z[0:64, 2 * hd:2 * hd + 1], in1=bvT[0:64, hd, 0:1],
                )
                nc.vector.tensor_add(
                    out=ctxT[64:128, hd, j:j + 1],
                    in0=ps_z[64:128, 2 * hd + 1:2 * hd + 2], in1=bvT[64:128, hd, 0:1],
                )

        probsT0 = scores_softmax(0, xT0)
        pxt0_ctx.close()  # free seq-0 xT before seq-1 pools
        px2 = stage1_ctx.enter_context(tc.tile_pool(name="px2", bufs=1))
        x1 = px2.tile([128, SC, H], F32R, name="x1")
        for sc in range(SC):
            nc.sync.dma_start(
                out=x1[:, sc, :],
                in_=_apr(feat.tensor, (S + sc * 128) * H, [[H, 128], [1, H]]),
            )
        pxt1 = stage1_ctx.enter_context(tc.tile_pool(name="pxt1", bufs=1))
        yt_zt(0, x0, probsT0)
        xT1 = build_xT(x1, pxt1)
        probsT1 = scores_softmax(1, xT1)
        yt_zt(1, x1, probsT1)

        # ---------------- row chain on the 2 CLS rows ----------------
        stage1_ctx.close()  # free wv/x/xT SBUF for the row chain
        with ExitStack() as c4:
            pwp = c4.enter_context(tc.tile_pool(name="pwp", bufs=1))
            prc = c4.enter_context(tc.tile_pool(name="prc", bufs=1))

            wp_sb = pwp.tile([128, HC, H], F32R)
            nc.gpsimd.dma_start(
                out=wp_sb, in_=_ap(io["wp"].tensor, 0, [[H, 128], [128 * H, HC], [1, H]])
            )

            def load_row_rc(name, n):
                t = prc.tile([1, n], F32R, name=f"{name}_row")
                nc.sync.dma_start(out=t, in_=_apr(io[name].tensor, 0, [[0, 1], [1, n]]))
                return t

            bo_row = load_row_rc("bo", H)
            b1_row = load_row_rc("b1", FF)
            b2_row = load_row_rc("b2", H)
            bp_row = load_row_rc("bp", H)
            bm_row = prc.tile([1, 2], F32R)
            nc.sync.dma_start(out=bm_row[0:1, 0:1], in_=_apr(io["bm"].tensor, 0, [[0, 1], [1, 1]]))
            nc.sync.dma_start(out=bm_row[0:1, 1:2], in_=_apr(io["bm"].tensor, 0, [[0, 1], [1, 1]]))

            def load_bcast(name, p, n):
                t = prc.tile([p, n], F32, name=f"{name}_bc")
                nc.sync.dma_start(out=t, in_=_ap(io[name].tensor, 0, [[0, p], [1, n]]))
                return t

            ln1g2 = load_bcast("ln1_g", BL, H)
            ln1b2 = load_bcast("ln1_b", BL, H)
            ln2g2 = load_bcast("ln2_g", BL, H)
            ln2b2 = load_bcast("ln2_b", BL, H)

            eps2 = prc.tile([BL, 1], F32)
            nc.vector.memset(eps2, EPS)

            wm_2 = prc.tile([BL, H], F32R)
            nc.sync.dma_start(out=wm_2, in_=_apr(io["wm"].tensor, 0, [[0, BL], [1, H]]))
            wm_col = prc.tile([128, HC, BL], F32R)
            for c in range(HC):
                pt = ppt.tile([128, BL], F32R, name="pt", tag="pt")
                nc.tensor.transpose(pt[:, :], wm_2[:, c * 128:(c + 1) * 128], ident[0:BL, 0:BL])
                nc.vector.tensor_copy(out=wm_col[:, c, :], in_=pt[:, :])

            def ln_norm(x_sb, g2, b2t, out_dtype_tile):
                # LayerNorm over free dim 768 on [2, 768]
                stats = prc.tile([BL, 3, 6], F32, name="ln_stats", bufs=2)
                xg = x_sb.rearrange("p (g d) -> p g d", g=3)
                for g in range(3):
                    nc.vector.bn_stats(out=stats[:, g, :], in_=xg[:, g, :])
                mv = prc.tile([BL, 2], F32, name="ln_mv", bufs=2)
                nc.vector.bn_aggr(out=mv, in_=stats)
                sd = prc.tile([BL, 1], F32, name="ln_sd", bufs=2)
                nc.scalar.activation(
                    out=sd, in_=mv[:, 1:2], func=mybir.ActivationFunctionType.Sqrt,
                    bias=eps2, scale=1.0,
                )
                rstd = prc.tile([BL, 1], F32, name="ln_rstd", bufs=2)
                nc.vector.reciprocal(out=rstd, in_=sd)
                nc.vector.tensor_scalar(
                    out=x_sb, in0=x_sb, scalar1=mv[:, 0:1], scalar2=rstd,
                    op0=mybir.AluOpType.subtract, op1=mybir.AluOpType.mult,
                )
                nc.vector.tensor_mul(out=x_sb, in0=x_sb, in1=g2)
                nc.vector.tensor_add(out=out_dtype_tile, in0=x_sb, in1=b2t)

            def transpose_rows(src, n_chunks, name):
                # [2, n*128] f32r -> [128, n, 2] f32r
                t = prc.tile([128, n_chunks, BL], F32R, name=name)
                for c in range(n_chunks):
                    pt = ppt.tile([128, BL], F32R, name="pt", tag="pt")
                    nc.tensor.transpose(
                        pt[:, :], src[:, c * 128:(c + 1) * 128], ident[0:BL, 0:BL]
                    )
                    if c % 2 == 0:
                        nc.vector.tensor_copy(out=t[:, c, :], in_=pt[:, :])
                    else:
                        nc.scalar.activation(
                            out=t[:, c, :], in_=pt[:, :],
                            func=mybir.ActivationFunctionType.Copy,
                        )
                return t

            # attn = ctx @ wo + bo + f0 ; LN1
            ps_a = [ppm.tile([BL, 512], F32, name="mm", tag="mm"),
                    ppm.tile([BL, 256], F32, name="mm", tag="mm")]
            for c in range(HC):
                nc.tensor.matmul(ps_a[0][:, :], ctxT[:, c, :], wo_sb[:, c, 0:512],
                                 start=(c == 0), stop=False)
                nc.tensor.matmul(ps_a[1][:, :], ctxT[:, c, :], wo_sb[:, c, 512:768],
                                 start=(c == 0), stop=False)
            nc.tensor.matmul(ps_a[0][:, :], ones_row[0:1, 0:BL], bo_row[0:1, 0:512],
                             start=False, stop=False)
            nc.tensor.matmul(ps_a[1][:, :], ones_row[0:1, 0:BL], bo_row[0:1, 512:768],
                             start=False, stop=False)
            nc.tensor.matmul(ps_a[0][:, :], ident[0:BL, 0:BL], f0_2[:, 0:512],
                             start=False, stop=True)
            nc.tensor.matmul(ps_a[1][:, :], ident[0:BL, 0:BL], f0_2[:, 512:768],
                             start=False, stop=True)

            attn_sb = prc.tile([BL, H], F32, name="attn_sb")
            nc.vector.tensor_copy(out=attn_sb[:, 0:512], in_=ps_a[0][:, :])
            nc.vector.tensor_copy(out=attn_sb[:, 512:768], in_=ps_a[1][:, :])
            A_sb = prc.tile([BL, H], F32R, name="A_sb")
            ln_norm(attn_sb, ln1g2, ln1b2, A_sb)
            AT = transpose_rows(A_sb, HC, "AT")

            # FFN1 + gelu: g = gelu(A @ w1 + b1); w1 streamed as column blocks
            g_sb = prc.tile([BL, FF], F32R, name="g_sb")
            for nb in range(FF // 256):
                w1_nb = pw1.tile([128, HC, 256], F32R, name="w1_nb")
                nc.sync.dma_start(
                    out=w1_nb,
                    in_=_apr(io["w1"].tensor, nb * 256,
                             [[FF, 128], [128 * FF, HC], [1, 256]]),
                )
                ps_h1 = ppm.tile([BL, 256], F32, name="mm", tag="mm")
                for c in range(HC):
                    nc.tensor.matmul(
                        ps_h1[:, :], AT[:, c, :], w1_nb[:, c, :],
                        start=(c == 0), stop=False,
                    )
                nc.tensor.matmul(
                    ps_h1[:, :], ones_row[0:1, 0:BL], b1_row[0:1, nb * 256:(nb + 1) * 256],
                    start=False, stop=True,
                )
                nc.scalar.activation(
                    out=g_sb[:, nb * 256:(nb + 1) * 256], in_=ps_h1[:, :],
                    func=mybir.ActivationFunctionType.Gelu,
                )
            gT = transpose_rows(g_sb, FFC, "gT")

            # FFN2 + residual: h2 = g @ w2 + b2 + A ; LN2
            ps_h2 = [ppm.tile([BL, 512], F32, name="mm", tag="mm"),
                     ppm.tile([BL, 256], F32, name="mm", tag="mm")]
            for c in range(FFC):
                w2_c = pw2.tile([128, H], F32R, name="w2_c")
                nc.sync.dma_start(
                    out=w2_c, in_=_apr(io["w2"].tensor, c * 128 * H, [[H, 128], [1, H]])
                )
                nc.tensor.matmul(ps_h2[0][:, :], gT[:, c, :], w2_c[:, 0:512],
                                 start=(c == 0), stop=False)
                nc.tensor.matmul(ps_h2[1][:, :], gT[:, c, :], w2_c[:, 512:768],
                                 start=(c == 0), stop=False)
            nc.tensor.matmul(ps_h2[0][:, :], ones_row[0:1, 0:BL], b2_row[0:1, 0:512],
                             start=False, stop=False)
            nc.tensor.matmul(ps_h2[1][:, :], ones_row[0:1, 0:BL], b2_row[0:1, 512:768],
                             start=False, stop=False)
            nc.tensor.matmul(ps_h2[0][:, :], ident[0:BL, 0:BL], A_sb[:, 0:512],
                             start=False, stop=True)
            nc.tensor.matmul(ps_h2[1][:, :], ident[0:BL, 0:BL], A_sb[:, 512:768],
                             start=False, stop=True)

            h2_sb = prc.tile([BL, H], F32, name="h2_sb")
            nc.vector.tensor_copy(out=h2_sb[:, 0:512], in_=ps_h2[0][:, :])
            nc.vector.tensor_copy(out=h2_sb[:, 512:768], in_=ps_h2[1][:, :])
            hid_sb = prc.tile([BL, H], F32R, name="hid_sb")
            ln_norm(h2_sb, ln2g2, ln2b2, hid_sb)
            hT = transpose_rows(hid_sb, HC, "hT")

            # pooler: pooled = tanh(hidden0 @ wp + bp)
            ps_p = [ppm.tile([BL, 512], F32, name="mm", tag="mm"),
                    ppm.tile([BL, 256], F32, name="mm", tag="mm")]
            for c in range(HC):
                nc.tensor.matmul(ps_p[0][:, :], hT[:, c, :], wp_sb[:, c, 0:512],
                                 start=(c == 0), stop=False)
                nc.tensor.matmul(ps_p[1][:, :], hT[:, c, :], wp_sb[:, c, 512:768],
                                 start=(c == 0), stop=False)
            nc.tensor.matmul(ps_p[0][:, :], ones_row[0:1, 0:BL], bp_row[0:1, 0:512],
                             start=False, stop=True)
            nc.tensor.matmul(ps_p[1][:, :], ones_row[0:1, 0:BL], bp_row[0:1, 512:768],
                             start=False, stop=True)
            pooled = prc.tile([BL, H], F32R, name="pooled")
            nc.scalar.activation(out=pooled[:, 0:512], in_=ps_p[0][:, :],
                                 func=mybir.ActivationFunctionType.Tanh)
            nc.scalar.activation(out=pooled[:, 512:768], in_=ps_p[1][:, :],
                                 func=mybir.ActivationFunctionType.Tanh)
            pT = transpose_rows(pooled, HC, "pT")

            # cls = pooled @ wm + bm  (N padded to 2 for fp32r evenness)
            ps_c = ppm.tile([BL, 2], F32, name="mm", tag="mm")
            for c in range(HC):
                nc.tensor.matmul(ps_c[:, :], pT[:, c, :], wm_col[:, c, :],
                                 start=(c == 0), stop=False)
            nc.tensor.matmul(ps_c[:, :], ones_row[0:1, 0:BL], bm_row[0:1, 0:2],
                             start=False, stop=True)
            out_sb = prc.tile([BL, 1], F32, name="out_sb")
            nc.vector.tensor_copy(out=out_sb, in_=ps_c[:, 0:1])
            nc.sync.dma_start(out=out[:, :], in_=out_sb)


_NC_CACHE = {}


def build_nc(repeat: int = 1):
    if repeat in _NC_CACHE:
        return _NC_CACHE[repeat]
    nc = bacc.Bacc("TRN2", target_bir_lowering=False, debug=False, num_devices=N_CORES)
    io = {}
    io["features"] = nc.dram_tensor("features", [BL, S, H], F32, kind="ExternalInput").ap()
    io["attention_mask"] = nc.dram_tensor("attention_mask", [BL, S], F32, kind="ExternalInput").ap()
    for nm, shape in [
        ("wq", [H, H]), ("wk", [H, H]), ("wv", [H, H]), ("wo", [H, H]),
        ("w1", [H, FF]), ("w2", [FF, H]), ("wp", [H, H]), ("wm", [H, 1]),
        ("bq", [H]), ("bk", [H]), ("bv", [H]), ("bo", [H]),
        ("b1", [FF]), ("b2", [H]), ("bp", [H]), ("bm", [1]),
        ("ln1_g", [H]), ("ln1_b", [H]), ("ln2_g", [H]), ("ln2_b", [H]),
    ]:
        io[nm] = nc.dram_tensor(nm, shape, F32, kind="ExternalInput").ap()
    io["out"] = nc.dram_tensor("out", [BL, 1], F32, kind="ExternalOutput").ap()

    with tile.TileContext(nc) as tc:
        bert_tile_kernel(tc, io, repeat=repeat)
    nc.compile()
    _NC_CACHE[repeat] = nc
    return nc


def kernel(**inputs) -> np.ndarray:
    nc = build_nc()
    weight_names = [
        "wq", "wk", "wv", "wo", "w1", "w2", "wp", "wm",
        "bq", "bk", "bv", "bo", "b1", "b2", "bp", "bm",
        "ln1_g", "ln1_b", "ln2_g", "ln2_b",
    ]
    shared = {nm: np.ascontiguousarray(np.asarray(inputs[nm], dtype=np.float32))
              for nm in weight_names}
    features = np.asarray(inputs["features"], dtype=np.float32)
    amask = np.asarray(inputs["attention_mask"], dtype=np.float32)

    in_maps = []
    for c in range(N_CORES):
        m = dict(shared)
        m["features"] = np.ascontiguousarray(features[c * BL:(c + 1) * BL])
        m["attention_mask"] = np.ascontiguousarray(amask[c * BL:(c + 1) * BL])
        in_maps.append(m)

    res = run_bass_kernel_spmd(nc, in_maps, core_ids=list(range(N_CORES)))
    return np.concatenate([res.results[c]["out"][:, 0] for c in range(N_CORES)])



# revision 2
# speedup vs baseline: 1.4660x; 1.4660x over previous
"""BERT interaction head on 8 trn2 NeuronCores.

Strategy (data-parallel, CLS-row folding, bf16 weights):
  - Batch 16 is sharded 2 sequences per core; each core runs the full head
    for its 2 sequences; host concatenates the 16 scalars.
  - Weights and features are cast to bf16 on the host (tolerance 2e-2 admits
    it): halves HBM traffic and enables the PE's automatic Fast Weight Load.
  - The output only depends on attention query row 0 (the CLS token), so the
    full Q/K/V projections are never materialized:
      scores_h = x @ (wk[:, h] @ q0_h) / sqrt(D)     (K never computed)
      ctx      = diag_blocks((probs @ x) @ wv) + bv  (V never computed)
    The bk term is constant per softmax row and cancels exactly.
  - Softmax statistics and LayerNorms stay in fp32.
  - Program order is arranged so seq-0's transpose work overlaps the weight
    DMAs and seq-1's feature load overlaps seq-0's attention.
"""

from contextlib import ExitStack

import numpy as np

import concourse.bacc as bacc
import concourse.bass as bass
import concourse.tile as tile
from concourse import mybir
from concourse._compat import with_exitstack
from concourse.bass_utils import run_bass_kernel_spmd
from concourse.masks import make_identity

F32 = mybir.dt.float32
BF16 = mybir.dt.bfloat16

B, S, H, NH, D, FF = 16, 1024, 768, 12, 64, 3072
N_CORES = 8
BL = B // N_CORES  # 2 sequences per core
HC = H // 128      # 6
SC = S // 128      # 8
FFC = FF // 128    # 24
EPS = 1e-12


def _ap(t, offset, dims):
    return bass.AP(tensor=t, offset=offset, ap=dims)


@with_exitstack
def bert_tile_kernel(ctx: ExitStack, tc: tile.TileContext, io: dict, repeat: int = 1):
    for _rep in range(repeat):
        _one_pass(tc, io)


def _one_pass(tc: tile.TileContext, io: dict):
    nc = tc.nc
    feat = io["features"]          # [2, 1024, 768] bf16
    amask = io["attention_mask"]   # [2, 1024] f32
    out = io["out"]                # [2, 1] f32

    with ExitStack() as ctx:
        # ---------------- pools (SBUF stack order matters) ----------------
        consts = ctx.enter_context(tc.tile_pool(name="consts", bufs=1))
        pwo = ctx.enter_context(tc.tile_pool(name="pwo", bufs=1))
        # FFN weight streams: alive from t=0 so their HWDGE transfers fill
        # every DMA gap during stage 1 (prefetch depth = pool size).
        pw1 = ctx.enter_context(tc.tile_pool(name="pw1", bufs=3))
        pw2 = ctx.enter_context(tc.tile_pool(name="pw2", bufs=2))
        stage1_ctx = ctx.enter_context(ExitStack())
        pwv = stage1_ctx.enter_context(tc.tile_pool(name="pwv", bufs=1))
        px = stage1_ctx.enter_context(tc.tile_pool(name="px", bufs=1))
        pxt0_ctx = stage1_ctx.enter_context(ExitStack())
        pxt = pxt0_ctx.enter_context(tc.tile_pool(name="pxt", bufs=1))

        ppt = ctx.enter_context(tc.tile_pool(name="ppt", bufs=4, space="PSUM"))
        ppm = ctx.enter_context(tc.tile_pool(name="ppm", bufs=2, space="PSUM"))
        pps = ctx.enter_context(tc.tile_pool(name="pps", bufs=2, space="PSUM"))

        # ---------------- identity first (gates all PE transposes) ----------
        ident_f = consts.tile([128, 128], F32)
        make_identity(nc, ident_f)
        ident = consts.tile([128, 128], BF16)
        nc.vector.tensor_copy(out=ident, in_=ident_f)

        ones_f = consts.tile([1, 16], F32)
        nc.vector.memset(ones_f, 1.0)
        ones_row = consts.tile([1, 16], BF16)
        nc.vector.tensor_copy(out=ones_row, in_=ones_f)

        # f0 rows (CLS features), plus transposed copy
        f0_2 = consts.tile([BL, H], BF16)
        nc.sync.dma_start(
            out=f0_2, in_=_ap(feat.tensor, 0, [[S * H, BL], [1, H]])
        )
        f0T = consts.tile([128, HC, BL], BF16)
        for c in range(HC):
            pt = ppt.tile([128, BL], BF16, name="pt", tag="pt")
            nc.tensor.transpose(pt[:, :], f0_2[:, c * 128:(c + 1) * 128], ident[0:BL, 0:BL])
            nc.vector.tensor_copy(out=f0T[:, c, :], in_=pt[:, :])

        def load_row_b(name, n):  # [1, n] bf16 dram -> bf16 sbuf row
            t = consts.tile([1, n], BF16, name=f"{name}_row")
            nc.sync.dma_start(out=t, in_=_ap(io[name].tensor, 0, [[0, 1], [1, n]]))
            return t

        bq_row = load_row_b("bq", H)

        # bv as f32 column stack via PE transpose (added post-PSUM in f32)
        bv_2 = consts.tile([BL, H], F32)
        nc.sync.dma_start(out=bv_2, in_=_ap(io["bv"].tensor, 0, [[0, BL], [1, H]]))

        # feature load for seq 0
        x0 = px.tile([128, SC, H], BF16, name="x0")
        nc.sync.dma_start(
            out=x0, in_=_ap(feat.tensor, 0, [[H, 128], [128 * H, SC], [1, H]])
        )
        bvT = consts.tile([128, HC, BL], F32)
        for c in range(HC):
            ptf = ppt.tile([128, BL], F32, name="ptf", tag="pt")
            nc.tensor.transpose(ptf[:, :], bv_2[:, c * 128:(c + 1) * 128], ident_f[0:BL, 0:BL])
            nc.vector.tensor_copy(out=bvT[:, c, :], in_=ptf[:, :])

        # stage-1 outputs
        ctxT = consts.tile([128, HC, BL], BF16)
        zeros_f = consts.tile([128, NH * BL], F32)
        nc.vector.memset(zeros_f, 0.0)
        q0bd = consts.tile([128, HC, NH * BL], BF16)
        for _c in range(HC):
            nc.vector.tensor_copy(out=q0bd[:, _c, :], in_=zeros_f)
        U_sb = consts.tile([128, HC, NH * BL], BF16)

        wv_sb = pwv.tile([128, HC, H], BF16)
        nc.gpsimd.dma_start(
            out=wv_sb, in_=_ap(io["wv"].tensor, 0, [[H, 128], [128 * H, HC], [1, H]])
        )
        # wo resident early so the row chain can start without waiting.
        # wv/wo ride the gpsimd (SWDGE) path: separate queue from the
        # latency-critical sync loads (x/wq/wk).
        wo_sb = pwo.tile([128, HC, H], BF16)
        nc.gpsimd.dma_start(
            out=wo_sb, in_=_ap(io["wo"].tensor, 0, [[H, 128], [128 * H, HC], [1, H]])
        )

        # ---- xT for seq 0: pure PE/DVE work overlapping the weight DMAs
        def build_xT(x_nat, pool=pxt):
            xT = pool.tile([128, HC, S], BF16, name="xT")
            for hc in range(HC):
                for sc in range(SC):
                    pt = ppt.tile([128, 128], BF16, name="pt", tag="pt")
                    nc.tensor.transpose(
                        pt[:, :], x_nat[:, sc, hc * 128:(hc + 1) * 128], ident[:, :]
                    )
                    dst = xT[:, hc, sc * 128:(sc + 1) * 128]
                    if (hc * SC + sc) % 2 == 0:
                        nc.vector.tensor_copy(out=dst, in_=pt[:, :])
                    else:
                        nc.scalar.activation(
                            out=dst, in_=pt[:, :],
                            func=mybir.ActivationFunctionType.Copy,
                        )
            return xT

        xT0 = build_xT(x0)

        # ---------------- q0 / wkT / U ----------------
        with tc.tile_pool(name="pwk_t", bufs=1) as pwkT:
            wkT_sb = pwkT.tile([128, HC, H], BF16)

            with tc.tile_pool(name="pwk_n", bufs=1) as pwkn:
                wk_nat = pwkn.tile([128, HC, H], BF16)
                nc.sync.dma_start(
                    out=wk_nat,
                    in_=_ap(io["wk"].tensor, 0, [[H, 128], [128 * H, HC], [1, H]]),
                )

                with tc.tile_pool(name="pwq", bufs=2) as pwq:
                    ps_q0 = [ppm.tile([BL, 512], F32, name="mm", tag="mm"),
                             ppm.tile([BL, 256], F32, name="mm", tag="mm")]
                    for c in range(HC):
                        wq_c = pwq.tile([128, H], BF16, name="wq_c")
                        nc.sync.dma_start(
                            out=wq_c,
                            in_=_ap(io["wq"].tensor, c * 128 * H, [[H, 128], [1, H]]),
                        )
                        nc.tensor.matmul(ps_q0[0][:, :], f0T[:, c, :], wq_c[:, 0:512],
                                         start=(c == 0), stop=False)
                        nc.tensor.matmul(ps_q0[1][:, :], f0T[:, c, :], wq_c[:, 512:768],
                                         start=(c == 0), stop=False)
                    nc.tensor.matmul(ps_q0[0][:, :], ones_row[0:1, 0:BL], bq_row[0:1, 0:512],
                                     start=False, stop=True)
                    nc.tensor.matmul(ps_q0[1][:, :], ones_row[0:1, 0:BL], bq_row[0:1, 512:768],
                                     start=False, stop=True)
                    q0_sb = consts.tile([BL, H], BF16)
                    nc.vector.tensor_copy(out=q0_sb[:, 0:512], in_=ps_q0[0][:, :])
                    nc.vector.tensor_copy(out=q0_sb[:, 512:768], in_=ps_q0[1][:, :])

                    # q0 block-diagonal, scaled by 1/sqrt(D)
                    # q0bd[p, c, 12*j + head] with head = 2c + p//64
                    for c in range(HC):
                        pt = ppt.tile([128, BL], BF16, name="pt", tag="pt")
                        nc.tensor.transpose(
                            pt[:, :], q0_sb[:, c * 128:(c + 1) * 128],
                            ident[0:BL, 0:BL],
                        )
                        for j in range(BL):
                            nc.vector.tensor_scalar_mul(
                                out=q0bd[0:64, c, NH * j + 2 * c: NH * j + 2 * c + 1],
                                in0=pt[0:64, j:j + 1], scalar1=1.0 / 8.0,
                            )
                            nc.vector.tensor_scalar_mul(
                                out=q0bd[64:128, c, NH * j + 2 * c + 1: NH * j + 2 * c + 2],
                                in0=pt[64:128, j:j + 1], scalar1=1.0 / 8.0,
                            )

                # wkT via PE transposes
                for c in range(HC):      # hh chunk of wk_nat
                    for d in range(HC):  # hd chunk
                        pt = ppt.tile([128, 128], BF16, name="pt", tag="pt")
                        nc.tensor.transpose(
                            pt[:, :], wk_nat[:, c, d * 128:(d + 1) * 128], ident[:, :]
                        )
                        dst = wkT_sb[:, d, c * 128:(c + 1) * 128]
                        if (c * HC + d) % 2 == 0:
                            nc.vector.tensor_copy(out=dst, in_=pt[:, :])
                        else:
                            nc.scalar.activation(
                                out=dst, in_=pt[:, :],
                                func=mybir.ActivationFunctionType.Copy,
                            )

            # U = wk^T-contracted q0bd (both sequences at once)
            for c in range(HC):  # hh chunk (output rows)
                ps_u = ppm.tile([128, NH * BL], F32, name="mm", tag="mm")
                for d in range(HC):  # hd chunk (contraction)
                    nc.tensor.matmul(
                        ps_u[:, :], wkT_sb[:, d, c * 128:(c + 1) * 128], q0bd[:, d, :],
                        start=(d == 0), stop=(d == HC - 1),
                    )
                if c % 2 == 0:
                    nc.vector.tensor_copy(out=U_sb[:, c, :], in_=ps_u[:, :])
                else:
                    nc.scalar.activation(
                        out=U_sb[:, c, :], in_=ps_u[:, :],
                        func=mybir.ActivationFunctionType.Copy,
                    )


        # ---------------- per-sequence attention ----------------
        def scores_softmax(j, xT):
            ps_s = [pps.tile([NH, 512], F32, name="ps_s", tag="ps_s"),
                    pps.tile([NH, 512], F32, name="ps_s", tag="ps_s")]
            for hc in range(HC):
                lhs = U_sb[:, hc, NH * j: NH * (j + 1)]
                nc.tensor.matmul(ps_s[0][:, :], lhs, xT[:, hc, 0:512],
                                 start=(hc == 0), stop=(hc == HC - 1))
                nc.tensor.matmul(ps_s[1][:, :], lhs, xT[:, hc, 512:1024],
                                 start=(hc == 0), stop=(hc == HC - 1))

            mask_bc = consts.tile([NH, S], F32, name="mask_bc", bufs=1)
            nc.sync.dma_start(
                out=mask_bc, in_=_ap(amask.tensor, j * S, [[0, NH], [1, S]])
            )
            scores = consts.tile([NH, S], F32, name="scores", bufs=1)
            nc.vector.tensor_add(out=scores[:, 0:512], in0=ps_s[0][:, :], in1=mask_bc[:, 0:512])
            nc.vector.tensor_add(out=scores[:, 512:1024], in0=ps_s[1][:, :], in1=mask_bc[:, 512:1024])

            negmax = consts.tile([NH, 1], F32, name="negmax", bufs=1)
            nc.vector.reduce_max(out=negmax, in_=scores, axis=mybir.AxisListType.X, negate=True)
            sumexp = consts.tile([NH, 1], F32, name="sumexp", bufs=1)
            probs = consts.tile([NH, S], BF16, name="probs", bufs=1)
            nc.scalar.activation(
                out=probs, in_=scores, func=mybir.ActivationFunctionType.Exp,
                bias=negmax, scale=1.0, accum_out=sumexp,
            )
            rec = consts.tile([NH, 1], F32, name="rec", bufs=1)
            nc.vector.reciprocal(out=rec, in_=sumexp)
            nc.vector.tensor_scalar_mul(out=probs, in0=probs, scalar1=rec)

            probsT = consts.tile([128, SC, NH], BF16, name="probsT", bufs=1)
            for sc in range(SC):
                pt = ppt.tile([128, NH], BF16, name="pt", tag="pt")
                nc.tensor.transpose(
                    pt[:, :], probs[:, sc * 128:(sc + 1) * 128], ident[0:NH, 0:NH]
                )
                if sc % 2 == 0:
                    nc.vector.tensor_copy(out=probsT[:, sc, :], in_=pt[:, :])
                else:
                    nc.scalar.activation(
                        out=probsT[:, sc, :], in_=pt[:, :],
                        func=mybir.ActivationFunctionType.Copy,
                    )
            return probsT

        def yt_zt(j, x_nat, probsT):
            # Y^T [hh, 12] = sum_s x^T probs^T  (lhsT = x blocks)
            YT_sb = consts.tile([128, HC, NH], BF16, name="YT_sb", bufs=1)
            for hc in range(HC):
                ps_y = ppm.tile([128, NH], F32, name="mm", tag="mm")
                for sc in range(SC):
                    nc.tensor.matmul(
                        ps_y[:, :], x_nat[:, sc, hc * 128:(hc + 1) * 128],
                        probsT[:, sc, :], start=(sc == 0), stop=(sc == SC - 1),
                    )
                if hc % 2 == 0:
                    nc.vector.tensor_copy(out=YT_sb[:, hc, :], in_=ps_y[:, :])
                else:
                    nc.scalar.activation(
                        out=YT_sb[:, hc, :], in_=ps_y[:, :],
                        func=mybir.ActivationFunctionType.Copy,
                    )

            # Z^T chunks [hd, 12]; diag-extract + bv -> ctxT[:, :, j]
            for hd in range(HC):
                ps_z = ppm.tile([128, NH], F32, name="mm", tag="mm")
                for hc in range(HC):
                    nc.tensor.matmul(
                        ps_z[:, :], wv_sb[:, hc, hd * 128:(hd + 1) * 128],
                        YT_sb[:, hc, :], start=(hc == 0), stop=(hc == HC - 1),
                    )
                nc.vector.tensor_add(
                    out=ctxT[0:64, hd, j:j + 1],
                    in0=ps_z[0:64, 2 * hd:2 * hd + 1], in1=bvT[0:64, hd, 0:1],
                )
                nc.vector.tensor_add(
                    out=ctxT[64:128, hd, j:j + 1],
                    in0=ps_z[64:128, 2 * hd + 1:2 * hd + 2], in1=bvT[64:128, hd, 0:1],
                )

        probsT0 = scores_softmax(0, xT0)
        pxt0_ctx.close()  # free seq-0 xT before seq-1 pools
        px2 = stage1_ctx.enter_context(tc.tile_pool(name="px2", bufs=1))
        x1 = px2.tile([128, SC, H], BF16, name="x1")
        nc.sync.dma_start(
            out=x1, in_=_ap(feat.tensor, S * H, [[H, 128], [128 * H, SC], [1, H]])
        )
        pxt1 = stage1_ctx.enter_context(tc.tile_pool(name="pxt1", bufs=1))
        yt_zt(0, x0, probsT0)
        xT1 = build_xT(x1, pxt1)
        probsT1 = scores_softmax(1, xT1)
        yt_zt(1, x1, probsT1)

        # ---------------- row chain on the 2 CLS rows ----------------
        stage1_ctx.close()  # free wv/x/xT SBUF for the row chain
        with ExitStack() as c4:
            pwp = c4.enter_context(tc.tile_pool(name="pwp", bufs=1))
            prc = c4.enter_context(tc.tile_pool(name="prc", bufs=1))

            wp_sb = pwp.tile([128, HC, H], BF16)
            nc.gpsimd.dma_start(
                out=wp_sb, in_=_ap(io["wp"].tensor, 0, [[H, 128], [128 * H, HC], [1, H]])
            )

            def load_row_rc(name, n):
                t = prc.tile([1, n], BF16, name=f"{name}_row")
                nc.sync.dma_start(out=t, in_=_ap(io[name].tensor, 0, [[0, 1], [1, n]]))
                return t

            bo_row = load_row_rc("bo", H)
            b1_row = load_row_rc("b1", FF)
            b2_row = load_row_rc("b2", H)
            bp_row = load_row_rc("bp", H)
            bm_row = prc.tile([1, 2], BF16)
            nc.sync.dma_start(out=bm_row[0:1, 0:1], in_=_ap(io["bm"].tensor, 0, [[0, 1], [1, 1]]))
            nc.sync.dma_start(out=bm_row[0:1, 1:2], in_=_ap(io["bm"].tensor, 0, [[0, 1], [1, 1]]))

            def load_bcast(name, p, n):
                t = prc.tile([p, n], F32, name=f"{name}_bc")
                nc.sync.dma_start(out=t, in_=_ap(io[name].tensor, 0, [[0, p], [1, n]]))
                return t

            ln1g2 = load_bcast("ln1_g", BL, H)
            ln1b2 = load_bcast("ln1_b", BL, H)
            ln2g2 = load_bcast("ln2_g", BL, H)
            ln2b2 = load_bcast("ln2_b", BL, H)

            eps2 = prc.tile([BL, 1], F32)
            nc.vector.memset(eps2, EPS)

            wm_2 = prc.tile([BL, H], BF16)
            nc.sync.dma_start(out=wm_2, in_=_ap(io["wm"].tensor, 0, [[0, BL], [1, H]]))
            wm_col = prc.tile([128, HC, BL], BF16)
            for c in range(HC):
                pt = ppt.tile([128, BL], BF16, name="pt", tag="pt")
                nc.tensor.transpose(pt[:, :], wm_2[:, c * 128:(c + 1) * 128], ident[0:BL, 0:BL])
                nc.vector.tensor_copy(out=wm_col[:, c, :], in_=pt[:, :])

            def ln_norm(x_sb, g2, b2t, out_dtype_tile):
                # LayerNorm over free dim 768 on [2, 768]
                stats = prc.tile([BL, 3, 6], F32, name="ln_stats", bufs=2)
                xg = x_sb.rearrange("p (g d) -> p g d", g=3)
                for g in range(3):
                    nc.vector.bn_stats(out=stats[:, g, :], in_=xg[:, g, :])
                mv = prc.tile([BL, 2], F32, name="ln_mv", bufs=2)
                nc.vector.bn_aggr(out=mv, in_=stats)
                sd = prc.tile([BL, 1], F32, name="ln_sd", bufs=2)
                nc.scalar.activation(
                    out=sd, in_=mv[:, 1:2], func=mybir.ActivationFunctionType.Sqrt,
                    bias=eps2, scale=1.0,
                )
                rstd = prc.tile([BL, 1], F32, name="ln_rstd", bufs=2)
                nc.vector.reciprocal(out=rstd, in_=sd)
                nc.vector.tensor_scalar(
                    out=x_sb, in0=x_sb, scalar1=mv[:, 0:1], scalar2=rstd,
                    op0=mybir.AluOpType.subtract, op1=mybir.AluOpType.mult,
                )
                nc.vector.tensor_mul(out=x_sb, in0=x_sb, in1=g2)
                nc.vector.tensor_add(out=out_dtype_tile, in0=x_sb, in1=b2t)

            def transpose_rows(src, n_chunks, name):
                # [2, n*128] bf16 -> [128, n, 2] bf16
                t = prc.tile([128, n_chunks, BL], BF16, name=name)
                for c in range(n_chunks):
                    pt = ppt.tile([128, BL], BF16, name="pt", tag="pt")
                    nc.tensor.transpose(
                        pt[:, :], src[:, c * 128:(c + 1) * 128], ident[0:BL, 0:BL]
                    )
                    if c % 2 == 0:
                        nc.vector.tensor_copy(out=t[:, c, :], in_=pt[:, :])
                    else:
                        nc.scalar.activation(
                            out=t[:, c, :], in_=pt[:, :],
                            func=mybir.ActivationFunctionType.Copy,
                        )
                return t

            # attn = ctx @ wo + bo + f0 ; LN1
            ps_a = [ppm.tile([BL, 512], F32, name="mm", tag="mm"),
                    ppm.tile([BL, 256], F32, name="mm", tag="mm")]
            for c in range(HC):
                nc.tensor.matmul(ps_a[0][:, :], ctxT[:, c, :], wo_sb[:, c, 0:512],
                                 start=(c == 0), stop=False)
                nc.tensor.matmul(ps_a[1][:, :], ctxT[:, c, :], wo_sb[:, c, 512:768],
                                 start=(c == 0), stop=False)
            nc.tensor.matmul(ps_a[0][:, :], ones_row[0:1, 0:BL], bo_row[0:1, 0:512],
                             start=False, stop=False)
            nc.tensor.matmul(ps_a[1][:, :], ones_row[0:1, 0:BL], bo_row[0:1, 512:768],
                             start=False, stop=False)
            nc.tensor.matmul(ps_a[0][:, :], ident[0:BL, 0:BL], f0_2[:, 0:512],
                             start=False, stop=True)
            nc.tensor.matmul(ps_a[1][:, :], ident[0:BL, 0:BL], f0_2[:, 512:768],
                             start=False, stop=True)

            attn_sb = prc.tile([BL, H], F32, name="attn_sb")
            nc.vector.tensor_copy(out=attn_sb[:, 0:512], in_=ps_a[0][:, :])
            nc.vector.tensor_copy(out=attn_sb[:, 512:768], in_=ps_a[1][:, :])
            A_sb = prc.tile([BL, H], BF16, name="A_sb")
            ln_norm(attn_sb, ln1g2, ln1b2, A_sb)
            AT = transpose_rows(A_sb, HC, "AT")

            # FFN1 + gelu: g = gelu(A @ w1 + b1); w1 streamed as column blocks
            g_sb = prc.tile([BL, FF], BF16, name="g_sb")
            for nb in range(FF // 512):
                w1_nb = pw1.tile([128, HC, 512], BF16, name="w1_nb")
                nc.sync.dma_start(
                    out=w1_nb,
                    in_=_ap(io["w1"].tensor, nb * 512,
                            [[FF, 128], [128 * FF, HC], [1, 512]]),
                )
                ps_h1 = ppm.tile([BL, 512], F32, name="mm", tag="mm")
                for c in range(HC):
                    nc.tensor.matmul(
                        ps_h1[:, :], AT[:, c, :], w1_nb[:, c, :],
                        start=(c == 0), stop=False,
                    )
                nc.tensor.matmul(
                    ps_h1[:, :], ones_row[0:1, 0:BL], b1_row[0:1, nb * 512:(nb + 1) * 512],
                    start=False, stop=True,
                )
                nc.scalar.activation(
                    out=g_sb[:, nb * 512:(nb + 1) * 512], in_=ps_h1[:, :],
                    func=mybir.ActivationFunctionType.Gelu,
                )
            gT = transpose_rows(g_sb, FFC, "gT")

            # FFN2 + residual: h2 = g @ w2 + b2 + A ; LN2
            ps_h2 = [ppm.tile([BL, 512], F32, name="mm", tag="mm"),
                     ppm.tile([BL, 256], F32, name="mm", tag="mm")]
            for grp in range(FFC // 6):
                w2_g = pw2.tile([128, 6, H], BF16, name="w2_g")
                nc.sync.dma_start(
                    out=w2_g,
                    in_=_ap(io["w2"].tensor, grp * 6 * 128 * H,
                            [[H, 128], [128 * H, 6], [1, H]]),
                )
                for c6 in range(6):
                    c = grp * 6 + c6
                    nc.tensor.matmul(ps_h2[0][:, :], gT[:, c, :], w2_g[:, c6, 0:512],
                                     start=(c == 0), stop=False)
                    nc.tensor.matmul(ps_h2[1][:, :], gT[:, c, :], w2_g[:, c6, 512:768],
                                     start=(c == 0), stop=False)
            nc.tensor.matmul(ps_h2[0][:, :], ones_row[0:1, 0:BL], b2_row[0:1, 0:512],
                             start=False, stop=False)
            nc.tensor.matmul(ps_h2[1][:, :], ones_row[0:1, 0:BL], b2_row[0:1, 512:768],
                             start=False, stop=False)
            nc.tensor.matmul(ps_h2[0][:, :], ident[0:BL, 0:BL], A_sb[:, 0:512],
                             start=False, stop=True)
            nc.tensor.matmul(ps_h2[1][:, :], ident[0:BL, 0:BL], A_sb[:, 512:768],
                             start=False, stop=True)

            h2_sb = prc.tile([BL, H], F32, name="h2_sb")
            nc.vector.tensor_copy(out=h2_sb[:, 0:512], in_=ps_h2[0][:, :])
            nc.vector.tensor_copy(out=h2_sb[:, 512:768], in_=ps_h2[1][:, :])
            hid_sb = prc.tile([BL, H], BF16, name="hid_sb")
            ln_norm(h2_sb, ln2g2, ln2b2, hid_sb)
            hT = transpose_rows(hid_sb, HC, "hT")

            # pooler: pooled = tanh(hidden0 @ wp + bp)
            ps_p = [ppm.tile([BL, 512], F32, name="mm", tag="mm"),
                    ppm.tile([BL, 256], F32, name="mm", tag="mm")]
            for c in range(HC):
                nc.tensor.matmul(ps_p[0][:, :], hT[:, c, :], wp_sb[:, c, 0:512],
                                 start=(c == 0), stop=False)
                nc.tensor.matmul(ps_p[1][:, :], hT[:, c, :], wp_sb[:, c, 512:768],
                                 start=(c == 0), stop=False)
            nc.tensor.matmul(ps_p[0][:, :], ones_row[0:1, 0:BL], bp_row[0:1, 0:512],
                             start=False, stop=True)
            nc.tensor.matmul(ps_p[1][:, :], ones_row[0:1, 0:BL], bp_row[0:1, 512:768],
                             start=False, stop=True)
            pooled = prc.tile([BL, H], BF16, name="pooled")
            nc.scalar.activation(out=pooled[:, 0:512], in_=ps_p[0][:, :],
                                 func=mybir.ActivationFunctionType.Tanh)
            nc.scalar.activation(out=pooled[:, 512:768], in_=ps_p[1][:, :],
                                 func=mybir.ActivationFunctionType.Tanh)
            pT = transpose_rows(pooled, HC, "pT")

            # cls = pooled @ wm + bm  (N padded to 2)
            ps_c = ppm.tile([BL, 2], F32, name="mm", tag="mm")
            for c in range(HC):
                nc.tensor.matmul(ps_c[:, :], pT[:, c, :], wm_col[:, c, :],
                                 start=(c == 0), stop=False)
            nc.tensor.matmul(ps_c[:, :], ones_row[0:1, 0:BL], bm_row[0:1, 0:2],
                             start=False, stop=True)
            out_sb = prc.tile([BL, 1], F32, name="out_sb")
            nc.vector.tensor_copy(out=out_sb, in_=ps_c[:, 0:1])
            nc.sync.dma_start(out=out[:, :], in_=out_sb)


_NC_CACHE = {}

BF16_NAMES = [
    "features", "wq", "wk", "wv_bf", "wo", "w1", "w2", "wp", "wm",
    "bq", "bo", "b1", "b2", "bp", "bm",
]


def build_nc(repeat: int = 1):
    if repeat in _NC_CACHE:
        return _NC_CACHE[repeat]
    nc = bacc.Bacc("TRN2", target_bir_lowering=False, debug=False, num_devices=N_CORES)
    io = {}
    io["features"] = nc.dram_tensor("features", [BL, S, H], BF16, kind="ExternalInput").ap()
    io["attention_mask"] = nc.dram_tensor("attention_mask", [BL, S], F32, kind="ExternalInput").ap()
    for nm, shape in [
        ("wq", [H, H]), ("wk", [H, H]), ("wv", [H, H]), ("wo", [H, H]),
        ("w1", [H, FF]), ("w2", [FF, H]), ("wp", [H, H]), ("wm", [H, 1]),
        ("bq", [H]), ("bo", [H]),
        ("b1", [FF]), ("b2", [H]), ("bp", [H]), ("bm", [1]),
    ]:
        io[nm] = nc.dram_tensor(nm, shape, BF16, kind="ExternalInput").ap()
    for nm, shape in [
        ("bv", [H]),
        ("ln1_g", [H]), ("ln1_b", [H]), ("ln2_g", [H]), ("ln2_b", [H]),
    ]:
        io[nm] = nc.dram_tensor(nm, shape, F32, kind="ExternalInput").ap()
    io["out"] = nc.dram_tensor("out", [BL, 1], F32, kind="ExternalOutput").ap()

    with tile.TileContext(nc) as tc:
        bert_tile_kernel(tc, io, repeat=repeat)
    nc.compile()
    _NC_CACHE[repeat] = nc
    return nc


def make_in_maps(inputs):
    import ml_dtypes
    bf = ml_dtypes.bfloat16
    bf_names = ["wq", "wk", "wv", "wo", "w1", "w2", "wp", "wm",
                "bq", "bo", "b1", "b2", "bp", "bm"]
    f32_names = ["bv", "ln1_g", "ln1_b", "ln2_g", "ln2_b"]
    shared = {nm: np.ascontiguousarray(np.asarray(inputs[nm], dtype=np.float32).astype(bf))
              for nm in bf_names}
    shared.update({nm: np.ascontiguousarray(np.asarray(inputs[nm], dtype=np.float32))
                   for nm in f32_names})
    features = np.asarray(inputs["features"], dtype=np.float32).astype(bf)
    amask = np.asarray(inputs["attention_mask"], dtype=np.float32)

    in_maps = []
    for c in range(N_CORES):
        m = dict(shared)
        m["features"] = np.ascontiguousarray(features[c * BL:(c + 1) * BL])
        m["attention_mask"] = np.ascontiguousarray(amask[c * BL:(c + 1) * BL])
        in_maps.append(m)
    return in_maps


def kernel(**inputs) -> np.ndarray:
    nc = build_nc()
    in_maps = make_in_maps(inputs)
    res = run_bass_kernel_spmd(nc, in_maps, core_ids=list(range(N_CORES)))
    return np.concatenate([res.results[c]["out"][:, 0] for c in range(N_CORES)])
